# revision 1
# baseline (speedup 1.0000x reference)
"""BoxRenderLoss Trainium2 kernel.

loss = mean over (box, fragment) pairs of masked min-squared-distance between
each box's 10x10 fragment grid and the other box's 100-point sampled boundary,
both directions, / (2*B*FP).

Algorithm: the min over the 100 boundary points decomposes into the 4 box
edges; each edge's 25-point uniform grid min has the closed form
k* = clamp(round(u/s), 0, 24), val = u - s*k*.  Per (row, fragment) item:
  dmin = min( min(ux^2, vx^2) + valy^2,  min(uy^2, vy^2) + valx^2 )
  mask = min(ux, vx, uy, vy) < 0         (fragment outside other box)
  contribution = dmin * mask

Device layout: partitions = 100 fragment points, free dim = virtual rows
(4096 boxes x 2 directions, data-parallel over 8 cores -> 1024 columns/core,
2 chunks of 512, x|y packed side by side -> FD 1024).  The affine maps
U = gx*w + dx, V = -gx*w + dvx, T = gx*(w*rix) + dx*rix and broadcast SB = s
are K<=2 outer-product matmuls on the TensorEngine.  All matmul operands live
in ONE SBUF tile (rows 0-1 / 32-33 / 64-65 for the base-partition-0/32/64
groups) and each PSUM tensor is consumed by exactly one engine -- PE Matmult
instructions only support a single semaphore wait.  Final per-partition row
sums come free via scalar_tensor_tensor's accum_out; host sums 100x2x8
partials and divides.
"""

import os
import numpy as np

# Exact float32 bit patterns of jnp.linspace(0.0, 1.0, 10) (fragment grid).
_LIN10 = np.array(
    [0, 1038323257, 1046711865, 1051372203, 1055100473,
     1057896676, 1059760811, 1061624946, 1063489081, 1065353216],
    dtype=np.uint32,
).view(np.float32)

_B = 4096
_FP = 100
_N_CORES = 8
_BOX_PER_CORE = _B // _N_CORES          # 512
_COLS = 2 * _BOX_PER_CORE               # 1024 virtual rows per core
_CHUNK = 512
_N_CHUNKS = _COLS // _CHUNK             # 2
_MAGIC = 8388608.0                      # 2^23 round-to-nearest trick

# mm-input tile column layout: [lhsT 0:128 | block0 | block1 | block2]
_LW = 128
_MMW = _LW + 3 * _COLS                  # 3200

LAST_RESULTS = None  # BassKernelResults of the most recent run (for test.py)

_compiled = {}


def _build_nc():
    import concourse.bass as bass
    import concourse.bacc as bacc
    import concourse.tile as tile
    from concourse import mybir

    f32 = mybir.dt.float32
    bf16 = mybir.dt.bfloat16
    Op = mybir.AluOpType
    Act = mybir.ActivationFunctionType

    nc = bacc.Bacc("TRN2", target_bir_lowering=False, debug=False,
                   num_devices=_N_CORES)
    f32r = mybir.dt.float32r
    mmin_d = nc.dram_tensor("mmin", [6, _MMW], f32r,
                            kind="ExternalInput").ap()
    out_d = nc.dram_tensor("out", [_FP, _N_CHUNKS], f32,
                           kind="ExternalOutput").ap()

    def blk(b, c):  # rhs slice columns for block b, chunk c
        start = _LW + b * _COLS + c * _CHUNK
        return slice(start, start + _CHUNK)

    from concourse.tile import add_dep_helper

    with tile.TileContext(nc) as tc:
        with (
            tc.tile_pool(name="const", bufs=1) as const,
            tc.tile_pool(name="sb", bufs=4) as sb,
            tc.tile_pool(name="ps", bufs=1, space="PSUM") as ps,
        ):
            mt = const.tile([66, _MMW], f32r)
            # One DMA per base-partition row group so every matmul carries at
            # most one new DMA wait.
            nc.sync.dma_start(mt[0:2, :], mmin_d[0:2, :])
            nc.sync.dma_start(mt[32:34, :], mmin_d[2:4, :])
            nc.sync.dma_start(mt[64:66, :], mmin_d[4:6, :])
            part = const.tile([_FP, _N_CHUNKS], f32)

            # float32r (same bits as f32): makes tile_legalize split each
            # Matmult into LdWeights + Matmult so semaphore waits spread
            # across two PE instructions (Matmult's LW slot fits only one).
            mtr = mt
            gx = mtr[0:2, :_FP]     # [gx; ones]
            gy = mtr[32:34, :_FP]   # [gy; ones]
            one = mtr[64:65, :_FP]  # [ones]

            for c in range(_N_CHUNKS):
                W = 2 * _CHUNK
                U = ps.tile([_FP, W], f32, tag="U")
                V = ps.tile([_FP, W], f32, tag="V")
                T = ps.tile([_FP, W], f32, tag="T")
                SB = ps.tile([_FP, W], f32, tag="SB")
                xh = slice(0, _CHUNK)
                yh = slice(_CHUNK, W)

                nc.tensor.matmul(U[:, xh], gx, mtr[0:2, blk(0, c)])
                nc.tensor.matmul(U[:, yh], gy, mtr[32:34, blk(0, c)])
                nc.tensor.matmul(V[:, xh], gx, mtr[0:2, blk(1, c)])
                nc.tensor.matmul(V[:, yh], gy, mtr[32:34, blk(1, c)])
                nc.tensor.matmul(T[:, xh], gx, mtr[0:2, blk(2, c)])
                nc.tensor.matmul(T[:, yh], gy, mtr[32:34, blk(2, c)])
                sxs = slice(_LW + c * _CHUNK, _LW + c * _CHUNK + _CHUNK)
                sys_ = slice(_LW + _COLS + c * _CHUNK,
                             _LW + _COLS + c * _CHUNK + _CHUNK)
                nc.tensor.matmul(SB[:, xh], one, mtr[64:65, sxs])
                nc.tensor.matmul(SB[:, yh], one, mtr[64:65, sys_])

                usq = sb.tile([_FP, W], bf16, tag="usq")
                nc.scalar.activation(usq[:], U[:], Act.Square)
                vs = sb.tile([_FP, W], f32, tag="vs")
                nc.scalar.activation(vs[:], V[:], Act.Copy)
                vsq = sb.tile([_FP, W], bf16, tag="vsq")
                nc.scalar.activation(vsq[:], V[:], Act.Square)

                r1 = sb.tile([_FP, W], bf16, tag="r1")
                nc.scalar.activation(r1[:], T[:], Act.Relu)
                kc1 = sb.tile([_FP, W], f32, tag="kc1")
                nc.vector.tensor_scalar(kc1[:], r1[:], _MAGIC,
                                        _MAGIC + 24.0, Op.add, Op.min)
                sk = sb.tile([_FP, W], f32, tag="sk")
                nc.vector.scalar_tensor_tensor(sk[:], kc1[:], _MAGIC, SB[:],
                                               Op.subtract, Op.mult)
                val = sb.tile([_FP, W], f32, tag="val")
                nc.vector.tensor_tensor(val[:], U[:], sk[:], Op.subtract)
                vq = sb.tile([_FP, W], bf16, tag="vq")
                nc.scalar.activation(vq[:], val[:], Act.Square)

                m1 = sb.tile([_FP, W], bf16, tag="m1")
                nc.vector.tensor_tensor(m1[:], U[:], vs[:], Op.min)
                mm = sb.tile([_FP, _CHUNK], bf16, tag="mm")
                nc.vector.tensor_tensor(mm[:], m1[:, xh], m1[:, yh], Op.min)

                exy = sb.tile([_FP, W], bf16, tag="exy")
                nc.vector.tensor_tensor(exy[:], usq[:], vsq[:], Op.min)
                e1 = sb.tile([_FP, _CHUNK], bf16, tag="e1")
                nc.vector.tensor_tensor(e1[:], exy[:, xh], vq[:, yh], Op.add)
                e2 = sb.tile([_FP, _CHUNK], bf16, tag="e2")
                nc.vector.tensor_tensor(e2[:], exy[:, yh], vq[:, xh], Op.add)
                dmin = sb.tile([_FP, _CHUNK], bf16, tag="dmin")
                nc.vector.tensor_tensor(dmin[:], e1[:], e2[:], Op.min)

                scr = sb.tile([_FP, _CHUNK], f32, tag="scr")
                nc.vector.scalar_tensor_tensor(
                    scr[:], mm[:], 0.0, dmin[:], Op.is_lt, Op.mult,
                    accum_out=part[:, c:c + 1])

            nc.sync.dma_start(out_d[:], part[:])
    nc.compile()
    return nc


def _combo_cols(A, T):
    """Per-row combo vectors (f32) for fragment-source boxes A vs grid
    boxes T.  Returns dict of [N] arrays."""
    A = A.astype(np.float32, copy=False)
    T = T.astype(np.float32, copy=False)
    w = A[:, 2] - A[:, 0]
    h = A[:, 3] - A[:, 1]
    tw = T[:, 2] - T[:, 0]
    th = T[:, 3] - T[:, 1]
    dx = A[:, 0] - T[:, 0]
    dy = A[:, 1] - T[:, 1]
    dvx = T[:, 2] - A[:, 0]
    dvy = T[:, 3] - A[:, 1]
    with np.errstate(divide="ignore"):
        rix = np.where(tw != 0, np.float32(24.0) / tw, np.float32(0.0))
        riy = np.where(th != 0, np.float32(24.0) / th, np.float32(0.0))
    rix = rix.astype(np.float32)
    riy = riy.astype(np.float32)
    return dict(
        w=w, dx=dx, nw=-w, dvx=dvx, wrx=w * rix, dxrx=dx * rix,
        h=h, dy=dy, nh=-h, dvy=dvy, wry=h * riy, dyry=dy * riy,
        sx=tw / np.float32(24.0), sy=th / np.float32(24.0))


def _mmin_for_core(boxes_c, targets_c):
    """Build the [6, _MMW] matmul-input DRAM tensor for one core."""
    d1 = _combo_cols(boxes_c, targets_c)     # dir1: cols 0:512
    d2 = _combo_cols(targets_c, boxes_c)     # dir2: cols 512:1024
    cat = {k: np.concatenate([d1[k], d2[k]]) for k in d1}

    m = np.zeros((6, _MMW), dtype=np.float32)
    m[0, :_FP] = np.repeat(_LIN10, 10)       # gx  (i of f = i*10+j)
    m[1, :_FP] = 1.0
    m[2, :_FP] = np.tile(_LIN10, 10)         # gy
    m[3, :_FP] = 1.0
    m[4, :_FP] = 1.0                         # ones (K=1 lhsT)
    B0, B1, B2 = (slice(_LW + b * _COLS, _LW + (b + 1) * _COLS)
                  for b in range(3))
    m[0, B0] = cat["w"];    m[1, B0] = cat["dx"]
    m[0, B1] = cat["nw"];   m[1, B1] = cat["dvx"]
    m[0, B2] = cat["wrx"];  m[1, B2] = cat["dxrx"]
    m[2, B0] = cat["h"];    m[3, B0] = cat["dy"]
    m[2, B1] = cat["nh"];   m[3, B1] = cat["dvy"]
    m[2, B2] = cat["wry"];  m[3, B2] = cat["dyry"]
    m[4, _LW:_LW + _COLS] = cat["sx"]
    m[4, _LW + _COLS:_LW + 2 * _COLS] = cat["sy"]
    return m


def kernel(boxes: np.ndarray, targets: np.ndarray) -> np.ndarray:
    from concourse.bass_utils import run_bass_kernel_spmd

    global LAST_RESULTS
    boxes = np.ascontiguousarray(boxes, dtype=np.float32)
    targets = np.ascontiguousarray(targets, dtype=np.float32)
    assert boxes.shape == (_B, 4) and targets.shape == (_B, 4)

    if "nc" not in _compiled:
        _compiled["nc"] = _build_nc()
    nc = _compiled["nc"]

    in_maps = []
    for c in range(_N_CORES):
        rows = slice(c * _BOX_PER_CORE, (c + 1) * _BOX_PER_CORE)
        in_maps.append({"mmin": _mmin_for_core(boxes[rows], targets[rows])})

    trace = bool(int(os.environ.get("BOXLOSS_TRACE", "0")))
    res = run_bass_kernel_spmd(nc, in_maps, list(range(_N_CORES)),
                               trace=trace)
    LAST_RESULTS = res

    total = np.float64(0.0)
    for r in res.results:
        total += r["out"].astype(np.float64).sum()
    loss = total / (2.0 * _B * _FP)
    return np.array(loss, dtype=np.float32)



# revision 4
# speedup vs baseline: 1.6171x; 1.6171x over previous
"""BoxRenderLoss Trainium2 kernel (rows-on-partitions layout).

loss = mean over (box, fragment) pairs of masked min-squared-distance between
each box's 10x10 fragment grid and the other box's 100-point sampled boundary,
both directions, / (2*B*FP).

Closed form: the min over the 100 boundary points decomposes into the 4 box
edges; each edge's 25-point uniform grid min is k* = clamp(round(u/s), 0, 24),
val = u - s*k*.  Per (row, i, j):
  dmin = min( ex_i + vqy_j,  ey_j + vqx_i )   where  ex = min(ux^2, vx^2),
  mask = min(mx_i, my_j) < 0                         vq = val^2,
  contribution = dmin * mask                         mx = min(ux, vx)

Everything per-row is 10-wide per coordinate (ux depends only on the x grid
index i), so the heavy chain runs on compact [128, 2*8*10] tiles: partitions
carry 128 row-groups, each partition holds 8 rows (row r = p*8 + s), x|y
side by side.  The (i,j) cross combine uses DVE broadcast access patterns
(step-0 dims) to expand 10-wide arrays to the 100 fragments without
materializing them: [128, 8*10*10] ops.  The mask op runs on GpSimd in
parallel with the DVE adds.  Final reduction: per-partition accum_out from
the masked-multiply, then one K=128 matmul against a ones column collapses
partitions to a single scalar per core; host sums 8 scalars / (2*B*FP).
"""

import os
import numpy as np

# Exact float32 bit patterns of jnp.linspace(0.0, 1.0, 10) (fragment grid).
_LIN10 = np.array(
    [0, 1038323257, 1046711865, 1051372203, 1055100473,
     1057896676, 1059760811, 1061624946, 1063489081, 1065353216],
    dtype=np.uint32,
).view(np.float32)

_B = 4096
_FP = 100
_N_CORES = 8
_BOX_PER_CORE = _B // _N_CORES          # 512
_ROWS = 2 * _BOX_PER_CORE               # 1024 virtual rows per core
_P = 128                                # partitions
_S = _ROWS // _P                        # 8 rows (slots) per partition
_MAGIC = 8388608.0                      # 2^23 round-to-nearest trick

# Input tile column layout (f32 cols):
#   G 0:20 (gx|gy), W 20:36 (w|h), D 36:52 (dx|dy), ONES 52:56,
#   TW 56:72 (tw|th), RI 72:88 (rix|riy), SS 88:104 (sx|sy)
_NCOL = 104
_CG, _CW, _CD, _C1, _CTW, _CRI, _CS = 0, 20, 36, 52, 56, 72, 88

LAST_RESULTS = None  # BassKernelResults of the most recent run (for test.py)

_compiled = {}


def _build_nc():
    import concourse.bass as bass
    import concourse.bacc as bacc
    import concourse.tile as tile
    from concourse import mybir

    f32 = mybir.dt.float32
    bf16 = mybir.dt.bfloat16
    Op = mybir.AluOpType
    Act = mybir.ActivationFunctionType

    nc = bacc.Bacc("TRN2", target_bir_lowering=False, debug=False,
                   num_devices=_N_CORES)
    in_d = nc.dram_tensor("inp", [_P, _NCOL], f32, kind="ExternalInput").ap()
    out_d = nc.dram_tensor("out", [1, 1], f32, kind="ExternalOutput").ap()

    CSI = [_P, 2, _S, 10]    # compact (coord, slot, grid) view
    XSIJ = [_P, _S, 10, 10]  # expanded (slot, i, j) view

    def cs(ap):   # [128, 160] dense -> (c, s, i)
        return ap.rearrange("p (c s i) -> p c s i", c=2, s=_S, i=10)

    def xi(ap):   # x half [128, 0:80] -> broadcast over j
        return (ap[:, 0:80].rearrange("p (s i) -> p s i", i=10)
                .unsqueeze(3).broadcast_to(XSIJ))

    def yj(ap):   # y half [128, 80:160] -> broadcast over i
        return (ap[:, 80:160].rearrange("p (s j) -> p s j", j=10)
                .unsqueeze(2).broadcast_to(XSIJ))

    def xe(ap):   # expanded [128, 800] dense -> (s, i, j)
        return ap.rearrange("p (s i j) -> p s i j", s=_S, i=10, j=10)

    with tile.TileContext(nc) as tc:
        with (
            tc.tile_pool(name="const", bufs=1) as const,
            tc.tile_pool(name="ps", bufs=1, space="PSUM") as ps,
        ):
            IN = const.tile([_P, _NCOL], f32)
            # Two DMAs on separate HWDGE queues (sync + scalar engines).
            nc.sync.dma_start(IN[:, 0:_CTW], in_d[:, 0:_CTW])
            nc.scalar.dma_start(IN[:, _CTW:_NCOL], in_d[:, _CTW:_NCOL])

            # Broadcast views of the per-row inputs.
            G = (IN[:, _CG:_CG + 20].rearrange("p (c i) -> p c i", i=10)
                 .unsqueeze(2).broadcast_to(CSI))
            W = (IN[:, _CW:_CW + 16].rearrange("p (c s) -> p c s", s=_S)
                 .unsqueeze(3).broadcast_to(CSI))
            D = (IN[:, _CD:_CD + 16].rearrange("p (c s) -> p c s", s=_S)
                 .unsqueeze(3).broadcast_to(CSI))
            TW = (IN[:, _CTW:_CTW + 16].rearrange("p (c s) -> p c s", s=_S)
                  .unsqueeze(3).broadcast_to(CSI))
            RI = (IN[:, _CRI:_CRI + 16].rearrange("p (c s) -> p c s", s=_S)
                  .unsqueeze(3).broadcast_to(CSI))
            SS = (IN[:, _CS:_CS + 16].rearrange("p (c s) -> p c s", s=_S)
                  .unsqueeze(3).broadcast_to(CSI))

            U = const.tile([_P, 160], f32)
            V = const.tile([_P, 160], f32)
            T = const.tile([_P, 160], f32)
            K = const.tile([_P, 160], f32)
            SK = const.tile([_P, 160], f32)
            VAL = const.tile([_P, 160], f32)
            M = const.tile([_P, 160], f32)
            UQ = const.tile([_P, 160], bf16)
            VQ2 = const.tile([_P, 160], bf16)
            EX = const.tile([_P, 160], bf16)
            VQ = const.tile([_P, 160], bf16)
            E1 = const.tile([_P, 800], bf16)
            E2 = const.tile([_P, 800], bf16)
            MM = const.tile([_P, 800], bf16)
            DM = const.tile([_P, 800], bf16)
            SCR = const.tile([_P, 800], bf16)
            part = const.tile([_P, 1], f32)
            outsb = const.tile([1, 1], f32)

            # Compact per-row precompute, x and y merged (FD=160).
            nc.vector.tensor_tensor(cs(T[:]), G, W, Op.mult)       # t0 = g*w
            nc.vector.tensor_tensor(cs(U[:]), cs(T[:]), D, Op.add)  # u
            nc.vector.tensor_tensor(cs(V[:]), TW, cs(U[:]), Op.subtract)
            nc.vector.tensor_tensor(cs(T[:]), cs(U[:]), RI, Op.mult)  # u/s
            nc.scalar.activation(UQ[:], U[:], Act.Square)
            nc.vector.tensor_scalar(K[:], T[:], 0.0, _MAGIC, Op.max, Op.add)
            nc.vector.tensor_scalar(K[:], K[:], _MAGIC + 24.0, _MAGIC,
                                    Op.min, Op.subtract)           # k*
            nc.vector.tensor_tensor(cs(SK[:]), cs(K[:]), SS, Op.mult)
            nc.vector.tensor_tensor(VAL[:], U[:], SK[:], Op.subtract)
            nc.vector.tensor_tensor(M[:], U[:], V[:], Op.min)      # mx|my
            nc.scalar.activation(VQ2[:], V[:], Act.Square)
            nc.scalar.activation(VQ[:], VAL[:], Act.Square)
            nc.vector.tensor_tensor(EX[:], UQ[:], VQ2[:], Op.min)  # ex|ey

            # (i, j) cross combine via broadcast APs.
            nc.vector.tensor_tensor(xe(MM[:]), xi(M[:]), yj(M[:]), Op.min)
            nc.vector.tensor_tensor(xe(E1[:]), xi(EX[:]), yj(VQ[:]), Op.add)
            nc.vector.tensor_tensor(xe(E2[:]), yj(EX[:]), xi(VQ[:]), Op.add)
            nc.vector.tensor_tensor(DM[:], E1[:], E2[:], Op.min)
            nc.vector.scalar_tensor_tensor(SCR[:], MM[:], 0.0, DM[:],
                                           Op.is_lt, Op.mult,
                                           accum_out=part[:])

            # Partition reduction: [128,1] x ones -> [1,1], then DMA out.
            pr = ps.tile([1, 1], f32)
            nc.tensor.matmul(pr[:], part[:], IN[:, _C1:_C1 + 1])
            nc.vector.tensor_copy(outsb[:], pr[:])
            nc.sync.dma_start(out_d[:], outsb[:])
    nc.compile()
    return nc


def _rows_for_core(boxes_c, targets_c):
    """Per-row input arrays for one core: dict of [1024] f32 arrays."""
    out = {}
    for name in ("w", "d", "tw", "ri", "ss"):
        out[name + "x"] = []
        out[name + "y"] = []
    for A, T in ((boxes_c, targets_c), (targets_c, boxes_c)):
        A = A.astype(np.float32, copy=False)
        T = T.astype(np.float32, copy=False)
        for axis, sfx in ((0, "x"), (1, "y")):
            w = A[:, 2 + axis] - A[:, 0 + axis]
            d = A[:, 0 + axis] - T[:, 0 + axis]
            tw = T[:, 2 + axis] - T[:, 0 + axis]
            with np.errstate(divide="ignore"):
                ri = np.where(tw != 0, np.float32(24.0) / tw, np.float32(0.0))
            out["w" + sfx].append(w)
            out["d" + sfx].append(d)
            out["tw" + sfx].append(tw)
            out["ri" + sfx].append(ri.astype(np.float32))
            out["ss" + sfx].append(tw / np.float32(24.0))
    return {k: np.concatenate(v).astype(np.float32) for k, v in out.items()}


def _input_for_core(boxes_c, targets_c):
    """Build the [128, 104] f32 input tile for one core."""
    r = _rows_for_core(boxes_c, targets_c)
    m = np.zeros((_P, _NCOL), dtype=np.float32)
    m[:, _CG:_CG + 10] = _LIN10
    m[:, _CG + 10:_CG + 20] = _LIN10
    m[:, _C1] = 1.0
    for base, (kx, ky) in (
        (_CW, ("wx", "wy")), (_CD, ("dx", "dy")), (_CTW, ("twx", "twy")),
        (_CRI, ("rix", "riy")), (_CS, ("ssx", "ssy")),
    ):
        m[:, base:base + _S] = r[kx].reshape(_P, _S)
        m[:, base + _S:base + 16] = r[ky].reshape(_P, _S)
    return m


def kernel(boxes: np.ndarray, targets: np.ndarray) -> np.ndarray:
    from concourse.bass_utils import run_bass_kernel_spmd

    global LAST_RESULTS
    boxes = np.ascontiguousarray(boxes, dtype=np.float32)
    targets = np.ascontiguousarray(targets, dtype=np.float32)
    assert boxes.shape == (_B, 4) and targets.shape == (_B, 4)

    if "nc" not in _compiled:
        _compiled["nc"] = _build_nc()
    nc = _compiled["nc"]

    in_maps = []
    for c in range(_N_CORES):
        rows = slice(c * _BOX_PER_CORE, (c + 1) * _BOX_PER_CORE)
        in_maps.append({"inp": _input_for_core(boxes[rows], targets[rows])})

    trace = bool(int(os.environ.get("BOXLOSS_TRACE", "0")))
    res = run_bass_kernel_spmd(nc, in_maps, list(range(_N_CORES)),
                               trace=trace)
    LAST_RESULTS = res

    total = np.float64(0.0)
    for r in res.results:
        total += np.float64(r["out"].reshape(()))
    loss = total / (2.0 * _B * _FP)
    return np.array(loss, dtype=np.float32)
